# revision 8
# baseline (speedup 1.0000x reference)
# Multi-head causal attention (B=4, S=2048, D=1024, H=16) on 8 TRN2 NeuronCores.
#
# Sharding: batch x query-chunk. Core c handles batch b=c//2 and two 512-row
# query chunks of that batch: cores with c%2==0 take real chunks (0, 3),
# c%2==1 take (1, 2). The SPMD program is identical on every core: it
# processes two query "slots" with fixed kk-tile capacities (8, 16); real
# chunk needs (4,8,12,16 tiles) are mapped into those capacities and the
# excess key tiles are zeroed by per-core causal-mask input data. Each core
# computes K/V projections for its whole batch (duplicated across the 2 cores
# sharing a batch) so no cross-core collectives are needed.
#
# All matmuls run as float32r (FP22-truncated fp32). Attention uses the
# transposed-scores layout St[kk, q] with Kt SBUF-resident:
#   Kt[d, s], Qt[d, q]; St = Kt_tile.T @ Qt  (2 heads packed into one 2-bank
#   PSUM tile, exp'd in a single ACT op)
#   P = exp(St) * mask
#   OT[dv, q] += V_aug[kk, 65].T @ P   -- V carries a ones column, so PSUM
#     row 64 accumulates the softmax denominators for free.
#   OT_norm = OT * reciprocal(bcast(denoms)); y = sum_dc OT.T @ woT + b_o.
import sys

if '/opt/trn_rl_repo' not in sys.path:
    sys.path.insert(0, '/opt/trn_rl_repo')

import numpy as np

B, S, D = 4, 2048, 1024
H, DK = 16, 64
NCORES = 8
SC = 512
NKT = S // 128            # 16 kk tiles
HPN = D // 128            # 8 head-pairs
CAPS = (8, 16)            # kk-tile capacity per slot (uniform across cores)
CHUNKS = [(0, 3), (1, 2)]  # real chunk pair per core parity

_CACHE = {}


def _build_program():
    import contextlib

    import concourse.tile as tile
    from concourse import bacc, mybir

    F32 = mybir.dt.float32
    F32R = mybir.dt.float32r
    BF16 = mybir.dt.bfloat16
    EXP = mybir.ActivationFunctionType.Exp

    nc = bacc.Bacc("TRN2", target_bir_lowering=False, debug=False,
                   num_devices=NCORES)

    xT_d = nc.dram_tensor("xT", [D, S], F32, kind="ExternalInput")
    xQT_d = nc.dram_tensor("xQT", [D, 2 * SC], F32, kind="ExternalInput")
    wqT_d = nc.dram_tensor("wqT", [D, D], F32, kind="ExternalInput")
    wkT_d = nc.dram_tensor("wkT", [D, D], F32, kind="ExternalInput")
    wvT_d = nc.dram_tensor("wvT", [D, D], F32, kind="ExternalInput")
    woT_d = nc.dram_tensor("woT", [D, D], F32, kind="ExternalInput")
    bias_d = nc.dram_tensor("bias", [1, D], F32, kind="ExternalInput")
    masks_d = nc.dram_tensor("masks", [128, NKT * SC], BF16,
                             kind="ExternalInput")
    y_d = nc.dram_tensor("y", [2 * SC, D], F32, kind="ExternalOutput")

    with tile.TileContext(nc) as tc, contextlib.ExitStack() as ctx:
        smalls = ctx.enter_context(tc.tile_pool(name="smalls", bufs=1))
        p_OT = ctx.enter_context(tc.tile_pool(name="otp", bufs=1))
        p_Kt = ctx.enter_context(tc.tile_pool(name="ktp", bufs=1))
        p_dram = ctx.enter_context(
            tc.tile_pool(name="dram", bufs=1, space="DRAM"))

        bias_sb = smalls.tile([1, D], F32R, tag="bias")
        nc.sync.dma_start(bias_sb[:], bias_d.ap().bitcast(F32R))
        ones1f = smalls.tile([1, 128], F32, tag="ones1f")
        nc.vector.memset(ones1f[:], 1.0)
        ones1 = smalls.tile([1, 128], F32R, tag="ones1")
        nc.vector.tensor_copy(ones1[:], ones1f[:])
        ones16f = smalls.tile([128, 16], F32, tag="ones16f")
        nc.vector.memset(ones16f[:], 1.0)
        ones16 = smalls.tile([128, 16], F32R, tag="ones16")
        nc.vector.tensor_copy(ones16[:], ones16f[:])

        OT = p_OT.tile([128, HPN * 2 * SC], F32R, tag="OT")
        Kt = p_Kt.tile([128, HPN * S], F32R, tag="Kt")

        vaug = p_dram.tile([128, NKT * H * 65], F32R, tag="vaug")
        Qtd = p_dram.tile([128, HPN * 2 * SC], F32R, tag="qtd")

        # ------------- V projection (xT streamed, wv resident) -----------
        with tc.tile_pool(name="wf", bufs=1) as p_w, \
             tc.tile_pool(name="xv", bufs=20) as p_xv, \
             tc.tile_pool(name="pb", bufs=4) as p_b, \
             tc.tile_pool(name="psp", bufs=6, space="PSUM") as psp:
            wv = p_w.tile([128, 8 * D], F32R, tag="w")
            for k in range(8):
                nc.sync.dma_start(
                    wv[:, k * D:(k + 1) * D],
                    wvT_d.ap()[k * 128:(k + 1) * 128, :].bitcast(F32R))
            for st in range(NKT):
                nc.sync.dma_start(
                    vaug[:, st * 1040:(st + 1) * 1040]
                    .rearrange("p (h c) -> p h c", c=65)[:, :, 64:65],
                    ones16[:].rearrange("p (h c) -> p h c", c=1))
                xv = []
                for k in range(8):
                    xt1 = p_xv.tile([128, 128], F32R, tag="xv",
                                    name=f"xv_{st}_{k}")
                    nc.sync.dma_start(
                        xt1[:],
                        xT_d.ap()[k * 128:(k + 1) * 128,
                                  st * 128:(st + 1) * 128].bitcast(F32R))
                    xv.append(xt1)
                for dvc in range(2):
                    ps = psp.tile([128, 512], F32, tag="ps")
                    for k in range(8):
                        nc.tensor.matmul(
                            ps[:], xv[k][:],
                            wv[:, k * D + dvc * 512:k * D + (dvc + 1) * 512],
                            start=(k == 0), stop=(k == 7))
                    vb = p_b.tile([128, 512], F32R, tag="vb")
                    nc.vector.tensor_copy(vb[:], ps[:])
                    off = st * 1040 + dvc * 520
                    nc.sync.dma_start(
                        vaug[:, off:off + 520]
                        .rearrange("p (h c) -> p h c", c=65)[:, :, 0:64],
                        vb[:].rearrange("p (h c) -> p h c", c=64))

        # ------------- Q projection (xQT streamed, wq resident) ----------
        with tc.tile_pool(name="wf2", bufs=1) as p_w2, \
             tc.tile_pool(name="xqs", bufs=4) as p_xq, \
             tc.tile_pool(name="pb2", bufs=4) as p_b2, \
             tc.tile_pool(name="psq", bufs=8, space="PSUM") as psq:
            wq = p_w2.tile([128, 8 * D], F32R, tag="w2")
            for k in range(8):
                nc.sync.dma_start(
                    wq[:, k * D:(k + 1) * D],
                    wqT_d.ap()[k * 128:(k + 1) * 128, :].bitcast(F32R))
            for ci in range(2):
                ps8 = [psq.tile([128, 512], F32, tag="ps",
                                name=f"psq_{ci}_{hp}") for hp in range(HPN)]
                for k in range(8):
                    xq1 = p_xq.tile([128, 512], F32R, tag="xq")
                    nc.sync.dma_start(
                        xq1[:],
                        xQT_d.ap()[k * 128:(k + 1) * 128,
                                   ci * SC:(ci + 1) * SC].bitcast(F32R))
                    for hp in range(HPN):
                        nc.tensor.matmul(
                            ps8[hp][:],
                            wq[:, k * D + hp * 128:k * D + (hp + 1) * 128],
                            xq1[:], start=(k == 0), stop=(k == 7))
                for hp in range(HPN):
                    qb = p_b2.tile([128, 512], F32R, tag="qb")
                    nc.vector.tensor_copy(qb[:], ps8[hp][:])
                    nc.sync.dma_start(
                        Qtd[:, hp * 2 * SC + ci * SC:
                            hp * 2 * SC + (ci + 1) * SC], qb[:])

        # ------------- K projection (xT streamed, wk resident) -----------
        # Kt written straight into its SBUF-resident tile.
        with tc.tile_pool(name="wf3", bufs=1) as p_w3, \
             tc.tile_pool(name="xrs", bufs=4) as p_xr, \
             tc.tile_pool(name="psk", bufs=8, space="PSUM") as psk:
            wk = p_w3.tile([128, 8 * D], F32R, tag="w3")
            for k in range(8):
                nc.sync.dma_start(
                    wk[:, k * D:(k + 1) * D],
                    wkT_d.ap()[k * 128:(k + 1) * 128, :].bitcast(F32R))
            for sc in range(4):
                ps8 = [psk.tile([128, 512], F32, tag="ps",
                                name=f"psk_{sc}_{hp}") for hp in range(HPN)]
                for k in range(8):
                    xr1 = p_xr.tile([128, 512], F32R, tag="xr")
                    nc.sync.dma_start(
                        xr1[:],
                        xT_d.ap()[k * 128:(k + 1) * 128,
                                  sc * 512:(sc + 1) * 512].bitcast(F32R))
                    for hp in range(HPN):
                        nc.tensor.matmul(
                            ps8[hp][:],
                            wk[:, k * D + hp * 128:k * D + (hp + 1) * 128],
                            xr1[:], start=(k == 0), stop=(k == 7))
                for hp in range(HPN):
                    nc.vector.tensor_copy(
                        Kt[:, hp * S + sc * 512:hp * S + (sc + 1) * 512],
                        ps8[hp][:])

        # ---------------- attention ----------------
        with tc.tile_pool(name="mk", bufs=1) as p_mk, \
             tc.tile_pool(name="rs", bufs=2) as p_rs, \
             tc.tile_pool(name="bcp", bufs=1) as p_bc, \
             tc.tile_pool(name="qts", bufs=3) as p_qt, \
             tc.tile_pool(name="vts", bufs=6) as p_vt, \
             tc.tile_pool(name="pp", bufs=4) as p_P, \
             tc.tile_pool(name="pst", bufs=2, space="PSUM") as p_st, \
             tc.tile_pool(name="pav", bufs=4, space="PSUM") as p_av:

            masks_sb = p_mk.tile([128, NKT * SC], BF16, tag="masks")
            nc.sync.dma_start(masks_sb[:], masks_d.ap())

            Qtv = Qtd[:].rearrange("p (hp q) -> p hp q", hp=HPN)

            for ci, cap in enumerate(CAPS):
                for bl in range(HPN // 2):
                    qt = p_qt.tile([128, 2 * SC], F32R, tag="qt")
                    nc.sync.dma_start(
                        qt[:].rearrange("p (a q) -> p a q", a=2),
                        Qtv[:, 2 * bl:2 * bl + 2, ci * SC:(ci + 1) * SC])
                    av = [p_av.tile([128, 512], F32, tag="av",
                                    name=f"av_{ci}_{bl}_{i}")
                          for i in range(4)]
                    for t in range(cap):
                        vt = p_vt.tile([128, 260], F32R, tag="vt")
                        nc.sync.dma_start(
                            vt[:],
                            vaug[:, t * 1040 + bl * 260:
                                 t * 1040 + (bl + 1) * 260])
                        for hp_i in range(2):
                            hp = 2 * bl + hp_i
                            st = p_st.tile([128, 1024], F32, tag="st")
                            for hh in range(2):
                                r0 = 64 * hh
                                nc.tensor.matmul(
                                    st[:, hh * 512:(hh + 1) * 512],
                                    Kt[r0:r0 + 64,
                                       hp * S + t * 128:hp * S + (t + 1) * 128],
                                    qt[r0:r0 + 64,
                                       hp_i * SC:(hp_i + 1) * SC],
                                    start=True, stop=True,
                                    tile_position=(r0, 0))
                            p1 = p_P.tile([128, 1024], F32R, tag="p")
                            nc.scalar.activation(p1[:], st[:], EXP)
                            if ci == 0 or t >= 8:
                                p2 = p_P.tile([128, 1024], F32R, tag="p")
                                for hh in range(2):
                                    nc.vector.tensor_mul(
                                        p2[:, hh * 512:(hh + 1) * 512],
                                        p1[:, hh * 512:(hh + 1) * 512],
                                        masks_sb[:, t * SC:(t + 1) * SC])
                                p1 = p2
                            for hh in range(2):
                                hi = 2 * hp_i + hh
                                nc.tensor.matmul(
                                    av[hi][0:65, :],
                                    vt[:, hi * 65:hi * 65 + 65],
                                    p1[:, hh * 512:(hh + 1) * 512],
                                    start=(t == 0), stop=(t == cap - 1))
                    # denominators -> broadcast -> reciprocal -> scale
                    rs = p_rs.tile([1, 2048], F32, tag="rs")
                    for hi in range(4):
                        nc.vector.tensor_copy(
                            rs[0:1, hi * 512:hi * 512 + 512],
                            av[hi][64:65, :])
                    bc = p_bc.tile([128, 2048], F32, tag="bc")
                    nc.gpsimd.partition_broadcast(bc[:], rs[:])
                    rbc = p_bc.tile([128, 2048], F32, tag="rbc")
                    scr = p_bc.tile([128, 2048], F32, tag="scr")
                    nc.vector.reciprocal_approx_accurate(
                        rbc[:], bc[:], scratch=scr[:])
                    for hi in range(4):
                        r0 = 64 * (hi % 2)
                        hp = 2 * bl + hi // 2
                        nc.vector.tensor_mul(
                            OT[r0:r0 + 64,
                               hp * 2 * SC + ci * SC:hp * 2 * SC + (ci + 1) * SC],
                            av[hi][0:64, :],
                            rbc[r0:r0 + 64, hi * 512:hi * 512 + 512])

        # ---------------- output projection ----------------
        with tc.tile_pool(name="wo", bufs=1) as p_wo, \
             tc.tile_pool(name="ybp", bufs=4) as p_yb, \
             tc.tile_pool(name="psy", bufs=6, space="PSUM") as psy:
            wo = p_wo.tile([128, 8 * D], F32R, tag="wo")
            for k in range(8):
                nc.sync.dma_start(
                    wo[:, k * D:(k + 1) * D],
                    woT_d.ap()[k * 128:(k + 1) * 128, :].bitcast(F32R))
            for qi in range(8):
                for nc2 in range(2):
                    ps = psy.tile([128, 512], F32, tag="psy")
                    for dc in range(8):
                        nc.tensor.matmul(
                            ps[:],
                            OT[:, dc * 2 * SC + qi * 128:
                               dc * 2 * SC + (qi + 1) * 128],
                            wo[:, dc * D + nc2 * 512:dc * D + (nc2 + 1) * 512],
                            start=(dc == 0), stop=False)
                    nc.tensor.matmul(
                        ps[:], ones1[:],
                        bias_sb[0:1, nc2 * 512:(nc2 + 1) * 512],
                        start=False, stop=True)
                    yb = p_yb.tile([128, 512], F32, tag="yb")
                    nc.vector.tensor_copy(yb[:], ps[:])
                    nc.sync.dma_start(
                        y_d.ap()[qi * 128:(qi + 1) * 128,
                                 nc2 * 512:(nc2 + 1) * 512], yb[:])

    nc.compile()
    return nc


def _get_program():
    if 'nc' not in _CACHE:
        _CACHE['nc'] = _build_program()
    return _CACHE['nc']


def _tri_masks():
    p = np.arange(128)[:, None]
    f = np.arange(SC)[None, :]
    return [(p <= f - 128 * r).astype(np.float32) for r in range(4)]


def _masks_for_core(c):
    import ml_dtypes
    tri = _tri_masks()
    ones = np.ones((128, SC), np.float32)
    zeros = np.zeros((128, SC), np.float32)
    j_pair = CHUNKS[c % 2]
    out = np.zeros((128, NKT * SC), np.float32)
    for ci, cap in enumerate(CAPS):
        j = j_pair[ci]
        t0 = 0 if ci == 0 else 8
        for t in range(t0, cap):
            if t < 4 * j:
                m = ones
            elif t < 4 * j + 4:
                m = tri[t - 4 * j]
            else:
                m = zeros
            out[:, t * SC:(t + 1) * SC] = m
    return out.astype(ml_dtypes.bfloat16)


def kernel(x, w_q, w_k, w_v, w_o, b_o):
    from concourse.bass_utils import run_bass_kernel_spmd

    x = np.asarray(x, dtype=np.float32)
    nc = _get_program()

    scale = np.float32(1.0 / np.sqrt(DK))
    common = {
        "wqT": np.ascontiguousarray(np.asarray(w_q, np.float32).T * scale),
        "wkT": np.ascontiguousarray(np.asarray(w_k, np.float32).T),
        "wvT": np.ascontiguousarray(np.asarray(w_v, np.float32).T),
        "woT": np.ascontiguousarray(np.asarray(w_o, np.float32).T),
        "bias": np.ascontiguousarray(np.asarray(b_o, np.float32)[None, :]),
    }

    in_maps = []
    for c in range(NCORES):
        b = c // 2
        j1, j2 = CHUNKS[c % 2]
        xb = x[b]
        xq = np.concatenate(
            [xb[j1 * SC:(j1 + 1) * SC], xb[j2 * SC:(j2 + 1) * SC]], axis=0)
        in_maps.append({
            "xT": np.ascontiguousarray(xb.T),
            "xQT": np.ascontiguousarray(xq.T),
            "masks": _masks_for_core(c),
            **common,
        })

    res = run_bass_kernel_spmd(nc, in_maps, core_ids=list(range(NCORES)),
                               trace=_CACHE.get('trace', False),
                               tmpdir=_CACHE.get('tmpdir'))
    _CACHE['last_res'] = res

    y = np.empty((B, S, D), dtype=np.float32)
    for c in range(NCORES):
        b = c // 2
        j1, j2 = CHUNKS[c % 2]
        yc = res.results[c]["y"]
        y[b, j1 * SC:(j1 + 1) * SC] = yc[0:SC]
        y[b, j2 * SC:(j2 + 1) * SC] = yc[SC:2 * SC]
    return y


# revision 9
# speedup vs baseline: 1.3899x; 1.3899x over previous
# Multi-head causal attention (B=4, S=2048, D=1024, H=16) on 8 TRN2 NeuronCores.
#
# Sharding: batch x query-chunk. Core c handles batch b=c//2 and two 512-row
# query chunks of that batch: cores with c%2==0 take real chunks (0, 3),
# c%2==1 take (1, 2). The SPMD program is identical on every core: it
# processes two query "slots" with fixed kk-tile capacities (8, 16); real
# chunk needs (4,8,12,16 tiles) are mapped into those capacities and the
# excess key tiles are zeroed by per-core causal-mask input data. Each core
# computes K/V projections for its whole batch (duplicated across the 2 cores
# sharing a batch) so no cross-core collectives are needed.
#
# All matmuls run as float32r (FP22-truncated fp32). Attention uses the
# transposed-scores layout St[kk, q] with Kt SBUF-resident:
#   Kt[d, s], Qt[d, q]; St = Kt_tile.T @ Qt  (2 heads packed into one 2-bank
#   PSUM tile, exp'd in a single ACT op)
#   P = exp(St) * mask
#   OT[dv, q] += V_aug[kk, 65].T @ P   -- V carries a ones column, so PSUM
#     row 64 accumulates the softmax denominators for free.
#   OT_norm = OT * reciprocal(bcast(denoms)); y = sum_dc OT.T @ woT + b_o.
import sys

if '/opt/trn_rl_repo' not in sys.path:
    sys.path.insert(0, '/opt/trn_rl_repo')

import numpy as np

B, S, D = 4, 2048, 1024
H, DK = 16, 64
NCORES = 8
SC = 512
NKT = S // 128            # 16 kk tiles
HPN = D // 128            # 8 head-pairs
CAPS = (8, 16)            # kk-tile capacity per slot (uniform across cores)
CHUNKS = [(0, 3), (1, 2)]  # real chunk pair per core parity

_CACHE = {}


def _build_program():
    import contextlib

    import concourse.tile as tile
    from concourse import bacc, mybir

    F32 = mybir.dt.float32
    F32R = mybir.dt.float32r
    BF16 = mybir.dt.bfloat16
    EXP = mybir.ActivationFunctionType.Exp

    nc = bacc.Bacc("TRN2", target_bir_lowering=False, debug=False,
                   num_devices=NCORES)

    xT_d = nc.dram_tensor("xT", [D, S], F32, kind="ExternalInput")
    xQT_d = nc.dram_tensor("xQT", [D, 2 * SC], F32, kind="ExternalInput")
    wqT_d = nc.dram_tensor("wqT", [D, D], F32, kind="ExternalInput")
    wkT_d = nc.dram_tensor("wkT", [D, D], F32, kind="ExternalInput")
    wvT_d = nc.dram_tensor("wvT", [D, D], F32, kind="ExternalInput")
    woT_d = nc.dram_tensor("woT", [D, D], F32, kind="ExternalInput")
    bias_d = nc.dram_tensor("bias", [1, D], F32, kind="ExternalInput")
    masks_d = nc.dram_tensor("masks", [128, NKT * 1024], BF16,
                             kind="ExternalInput")
    y_d = nc.dram_tensor("y", [2 * SC, D], F32, kind="ExternalOutput")

    with tile.TileContext(nc) as tc, contextlib.ExitStack() as ctx:
        smalls = ctx.enter_context(tc.tile_pool(name="smalls", bufs=1))
        p_OT = ctx.enter_context(tc.tile_pool(name="otp", bufs=1))
        p_Kt = ctx.enter_context(tc.tile_pool(name="ktp", bufs=1))
        p_dram = ctx.enter_context(
            tc.tile_pool(name="dram", bufs=1, space="DRAM"))

        bias_sb = smalls.tile([1, D], F32R, tag="bias")
        nc.sync.dma_start(bias_sb[:], bias_d.ap().bitcast(F32R))
        ones1f = smalls.tile([1, 128], F32, tag="ones1f")
        nc.vector.memset(ones1f[:], 1.0)
        ones1 = smalls.tile([1, 128], F32R, tag="ones1")
        nc.vector.tensor_copy(ones1[:], ones1f[:])
        ones256f = smalls.tile([128, 256], F32, tag="ones256f")
        nc.vector.memset(ones256f[:], 1.0)
        ones256 = smalls.tile([128, 256], F32R, tag="ones256")
        nc.vector.tensor_copy(ones256[:], ones256f[:])

        OT = p_OT.tile([128, HPN * 2 * SC], F32R, tag="OT")
        Kt = p_Kt.tile([128, HPN * S], F32R, tag="Kt")

        vaug = p_dram.tile([128, NKT * H * 65], F32R, tag="vaug")
        Qtd = p_dram.tile([128, HPN * 2 * SC], F32R, tag="qtd")

        # ones columns of vaug, all 16 s-tiles in one strided DMA
        nc.sync.dma_start(
            vaug[:].rearrange("p (s h c) -> p s h c", s=NKT, c=65)
            [:, :, :, 64:65],
            ones256[:].rearrange("p (s h) -> p s h", s=NKT)[:, :, :, None])

        # ---- V + K projections, one half of the sequence at a time ----
        with tc.tile_pool(name="xth", bufs=1) as p_xh, \
             tc.tile_pool(name="wfv", bufs=1) as p_wv, \
             tc.tile_pool(name="wfk", bufs=1) as p_wk, \
             tc.tile_pool(name="pb", bufs=2) as p_b, \
             tc.tile_pool(name="psp", bufs=8, space="PSUM") as psp:
            wv = p_wv.tile([128, 8 * D], F32R, tag="wv")
            for k in range(8):
                nc.sync.dma_start(
                    wv[:, k * D:(k + 1) * D],
                    wvT_d.ap()[k * 128:(k + 1) * 128, :].bitcast(F32R))
            wk = p_wk.tile([128, 8 * D], F32R, tag="wk")
            for k in range(8):
                nc.sync.dma_start(
                    wk[:, k * D:(k + 1) * D],
                    wkT_d.ap()[k * 128:(k + 1) * 128, :].bitcast(F32R))

            for half in range(2):
                xh = p_xh.tile([128, 8 * 1024], F32R, tag="xh",
                               name=f"xh_{half}")
                for k in range(8):
                    nc.sync.dma_start(
                        xh[:, k * 1024:(k + 1) * 1024],
                        xT_d.ap()[k * 128:(k + 1) * 128,
                                  half * 1024:(half + 1) * 1024]
                        .bitcast(F32R))
                # V for the 8 s-tiles of this half
                for sti in range(8):
                    st_g = half * 8 + sti
                    for dvc in range(2):
                        ps = psp.tile([128, 512], F32, tag="ps")
                        for k in range(8):
                            nc.tensor.matmul(
                                ps[:],
                                xh[:, k * 1024 + sti * 128:
                                   k * 1024 + (sti + 1) * 128],
                                wv[:, k * D + dvc * 512:k * D + (dvc + 1) * 512],
                                start=(k == 0), stop=(k == 7))
                        vb = p_b.tile([128, 512], F32R, tag="vb")
                        nc.scalar.copy(vb[:], ps[:])
                        off = st_g * 1040 + dvc * 520
                        nc.sync.dma_start(
                            vaug[:, off:off + 520]
                            .rearrange("p (h c) -> p h c", c=65)[:, :, 0:64],
                            vb[:].rearrange("p (h c) -> p h c", c=64))
                # K for the 2 s-chunks of this half -> SBUF-resident Kt
                for sc2 in range(2):
                    sc = half * 2 + sc2
                    ps8 = [psp.tile([128, 512], F32, tag="ps",
                                    name=f"psk_{sc}_{hp}")
                           for hp in range(HPN)]
                    for k in range(8):
                        for hp in range(HPN):
                            nc.tensor.matmul(
                                ps8[hp][:],
                                wk[:, k * D + hp * 128:k * D + (hp + 1) * 128],
                                xh[:, k * 1024 + sc2 * 512:
                                   k * 1024 + (sc2 + 1) * 512],
                                start=(k == 0), stop=(k == 7))
                    for hp in range(HPN):
                        nc.vector.tensor_copy(
                            Kt[:, hp * S + sc * 512:hp * S + (sc + 1) * 512],
                            ps8[hp][:])

        # ------------- Q projection (xQT streamed, wq resident) ----------
        with tc.tile_pool(name="wf2", bufs=1) as p_w2, \
             tc.tile_pool(name="xqs", bufs=4) as p_xq, \
             tc.tile_pool(name="pb2", bufs=4) as p_b2, \
             tc.tile_pool(name="psq", bufs=8, space="PSUM") as psq:
            wq = p_w2.tile([128, 8 * D], F32R, tag="w2")
            for k in range(8):
                nc.sync.dma_start(
                    wq[:, k * D:(k + 1) * D],
                    wqT_d.ap()[k * 128:(k + 1) * 128, :].bitcast(F32R))
            for ci in range(2):
                ps8 = [psq.tile([128, 512], F32, tag="ps",
                                name=f"psq_{ci}_{hp}") for hp in range(HPN)]
                for k in range(8):
                    xq1 = p_xq.tile([128, 512], F32R, tag="xq")
                    nc.sync.dma_start(
                        xq1[:],
                        xQT_d.ap()[k * 128:(k + 1) * 128,
                                   ci * SC:(ci + 1) * SC].bitcast(F32R))
                    for hp in range(HPN):
                        nc.tensor.matmul(
                            ps8[hp][:],
                            wq[:, k * D + hp * 128:k * D + (hp + 1) * 128],
                            xq1[:], start=(k == 0), stop=(k == 7))
                for hp in range(HPN):
                    qb = p_b2.tile([128, 512], F32R, tag="qb")
                    nc.scalar.copy(qb[:], ps8[hp][:])
                    nc.sync.dma_start(
                        Qtd[:, hp * 2 * SC + ci * SC:
                            hp * 2 * SC + (ci + 1) * SC], qb[:])

        # ---------------- attention ----------------
        with tc.tile_pool(name="mk", bufs=1) as p_mk, \
             tc.tile_pool(name="rs", bufs=2) as p_rs, \
             tc.tile_pool(name="bcp", bufs=1) as p_bc, \
             tc.tile_pool(name="qts", bufs=3) as p_qt, \
             tc.tile_pool(name="vts", bufs=3) as p_vt, \
             tc.tile_pool(name="pp", bufs=4) as p_P, \
             tc.tile_pool(name="pst", bufs=2, space="PSUM") as p_st, \
             tc.tile_pool(name="pav", bufs=4, space="PSUM") as p_av:

            masks_sb = p_mk.tile([128, NKT * 1024], BF16, tag="masks")
            nc.gpsimd.dma_start(masks_sb[:], masks_d.ap())

            Qtv = Qtd[:].rearrange("p (hp q) -> p hp q", hp=HPN)
            Vv = vaug[:].rearrange("p (t c) -> p t c", t=NKT)

            for ci, cap in enumerate(CAPS):
                for bl in range(HPN // 2):
                    qt = p_qt.tile([128, 2 * SC], F32R, tag="qt")
                    nc.gpsimd.dma_start(
                        qt[:].rearrange("p (a q) -> p a q", a=2),
                        Qtv[:, 2 * bl:2 * bl + 2, ci * SC:(ci + 1) * SC])
                    av = [p_av.tile([128, 512], F32, tag="av",
                                    name=f"av_{ci}_{bl}_{i}")
                          for i in range(4)]
                    vts = []
                    for tg in range(cap // 4):  # 4 kk-tiles per vt load
                        vt = p_vt.tile([128, 4 * 260], F32R, tag="vt",
                                       name=f"vt_{ci}_{bl}_{tg}")
                        nc.gpsimd.dma_start(
                            vt[:].rearrange("p (t c) -> p t c", t=4),
                            Vv[:, 4 * tg:4 * tg + 4,
                               bl * 260:(bl + 1) * 260])
                        vts.append(vt)
                    for t in range(cap):
                        vt_s = vts[t // 4][:, (t % 4) * 260:(t % 4) * 260 + 260]
                        for hp_i in range(2):
                            hp = 2 * bl + hp_i
                            st = p_st.tile([128, 1024], F32, tag="st")
                            for hh in range(2):
                                r0 = 64 * hh
                                nc.tensor.matmul(
                                    st[:, hh * 512:(hh + 1) * 512],
                                    Kt[r0:r0 + 64,
                                       hp * S + t * 128:hp * S + (t + 1) * 128],
                                    qt[r0:r0 + 64,
                                       hp_i * SC:(hp_i + 1) * SC],
                                    start=True, stop=True,
                                    tile_position=(r0, 0))
                            p1 = p_P.tile([128, 1024], F32R, tag="p")
                            nc.scalar.activation(p1[:], st[:], EXP)
                            if ci == 0 or t >= 8:
                                p2 = p_P.tile([128, 1024], F32R, tag="p")
                                nc.vector.tensor_mul(
                                    p2[:], p1[:],
                                    masks_sb[:, t * 1024:(t + 1) * 1024])
                                p1 = p2
                            for hh in range(2):
                                hi = 2 * hp_i + hh
                                nc.tensor.matmul(
                                    av[hi][0:65, :],
                                    vt_s[:, hi * 65:hi * 65 + 65],
                                    p1[:, hh * 512:(hh + 1) * 512],
                                    start=(t == 0), stop=(t == cap - 1))
                    # normalize, one head-pair at a time
                    for hp_i in range(2):
                        hp = 2 * bl + hp_i
                        rs = p_rs.tile([1, 1024], F32, tag="rs")
                        for hh in range(2):
                            hi = 2 * hp_i + hh
                            nc.vector.tensor_copy(
                                rs[0:1, hh * 512:hh * 512 + 512],
                                av[hi][64:65, :])
                        bc = p_bc.tile([128, 1024], F32, tag="bc")
                        nc.gpsimd.partition_broadcast(bc[:], rs[:])
                        rbc = p_bc.tile([128, 1024], F32, tag="rbc")
                        scr = p_bc.tile([128, 1024], F32, tag="scr")
                        nc.vector.reciprocal_approx_accurate(
                            rbc[:], bc[:], scratch=scr[:])
                        for hh in range(2):
                            hi = 2 * hp_i + hh
                            r0 = 64 * hh
                            nc.vector.tensor_mul(
                                OT[r0:r0 + 64,
                                   hp * 2 * SC + ci * SC:
                                   hp * 2 * SC + (ci + 1) * SC],
                                av[hi][0:64, :],
                                rbc[r0:r0 + 64, hh * 512:hh * 512 + 512])

        # ---------------- output projection ----------------
        with tc.tile_pool(name="wo", bufs=1) as p_wo, \
             tc.tile_pool(name="ybp", bufs=4) as p_yb, \
             tc.tile_pool(name="psy", bufs=6, space="PSUM") as psy:
            wo = p_wo.tile([128, 8 * D], F32R, tag="wo")
            for k in range(8):
                nc.sync.dma_start(
                    wo[:, k * D:(k + 1) * D],
                    woT_d.ap()[k * 128:(k + 1) * 128, :].bitcast(F32R))
            for qi in range(8):
                for nc2 in range(2):
                    ps = psy.tile([128, 512], F32, tag="psy")
                    for dc in range(8):
                        nc.tensor.matmul(
                            ps[:],
                            OT[:, dc * 2 * SC + qi * 128:
                               dc * 2 * SC + (qi + 1) * 128],
                            wo[:, dc * D + nc2 * 512:dc * D + (nc2 + 1) * 512],
                            start=(dc == 0), stop=False)
                    nc.tensor.matmul(
                        ps[:], ones1[:],
                        bias_sb[0:1, nc2 * 512:(nc2 + 1) * 512],
                        start=False, stop=True)
                    yb = p_yb.tile([128, 512], F32, tag="yb")
                    nc.vector.tensor_copy(yb[:], ps[:])
                    nc.sync.dma_start(
                        y_d.ap()[qi * 128:(qi + 1) * 128,
                                 nc2 * 512:(nc2 + 1) * 512], yb[:])

    nc.compile()
    return nc


def _get_program():
    if 'nc' not in _CACHE:
        _CACHE['nc'] = _build_program()
    return _CACHE['nc']


def _tri_masks():
    p = np.arange(128)[:, None]
    f = np.arange(SC)[None, :]
    return [(p <= f - 128 * r).astype(np.float32) for r in range(4)]


def _masks_for_core(c):
    import ml_dtypes
    tri = _tri_masks()
    ones = np.ones((128, SC), np.float32)
    zeros = np.zeros((128, SC), np.float32)
    j_pair = CHUNKS[c % 2]
    out = np.zeros((128, NKT * 1024), np.float32)
    for ci, cap in enumerate(CAPS):
        j = j_pair[ci]
        t0 = 0 if ci == 0 else 8
        for t in range(t0, cap):
            if t < 4 * j:
                m = ones
            elif t < 4 * j + 4:
                m = tri[t - 4 * j]
            else:
                m = zeros
            # both heads of a pair share the same [128, 512] mask
            out[:, t * 1024:t * 1024 + 512] = m
            out[:, t * 1024 + 512:(t + 1) * 1024] = m
    return out.astype(ml_dtypes.bfloat16)


def kernel(x, w_q, w_k, w_v, w_o, b_o):
    from concourse.bass_utils import run_bass_kernel_spmd

    x = np.asarray(x, dtype=np.float32)
    nc = _get_program()

    scale = np.float32(1.0 / np.sqrt(DK))
    common = {
        "wqT": np.ascontiguousarray(np.asarray(w_q, np.float32).T * scale),
        "wkT": np.ascontiguousarray(np.asarray(w_k, np.float32).T),
        "wvT": np.ascontiguousarray(np.asarray(w_v, np.float32).T),
        "woT": np.ascontiguousarray(np.asarray(w_o, np.float32).T),
        "bias": np.ascontiguousarray(np.asarray(b_o, np.float32)[None, :]),
    }

    in_maps = []
    for c in range(NCORES):
        b = c // 2
        j1, j2 = CHUNKS[c % 2]
        xb = x[b]
        xq = np.concatenate(
            [xb[j1 * SC:(j1 + 1) * SC], xb[j2 * SC:(j2 + 1) * SC]], axis=0)
        in_maps.append({
            "xT": np.ascontiguousarray(xb.T),
            "xQT": np.ascontiguousarray(xq.T),
            "masks": _masks_for_core(c),
            **common,
        })

    res = run_bass_kernel_spmd(nc, in_maps, core_ids=list(range(NCORES)),
                               trace=_CACHE.get('trace', False),
                               tmpdir=_CACHE.get('tmpdir'))
    _CACHE['last_res'] = res

    y = np.empty((B, S, D), dtype=np.float32)
    for c in range(NCORES):
        b = c // 2
        j1, j2 = CHUNKS[c % 2]
        yc = res.results[c]["y"]
        y[b, j1 * SC:(j1 + 1) * SC] = yc[0:SC]
        y[b, j2 * SC:(j2 + 1) * SC] = yc[SC:2 * SC]
    return y


# revision 10
# speedup vs baseline: 2.1405x; 1.5400x over previous
# Multi-head causal attention (B=4, S=2048, D=1024, H=16) on 8 TRN2 NeuronCores.
#
# Sharding: batch x query-chunk. Core c handles batch b=c//2 and two 512-row
# query chunks of that batch: cores with c%2==0 take real chunks (0, 3),
# c%2==1 take (1, 2). The SPMD program is identical on every core: it
# processes two query "slots" with fixed kk-tile capacities (8, 16); real
# chunk needs (4,8,12,16 tiles) are mapped into those capacities and the
# excess key tiles are zeroed by per-core causal-mask input data. Each core
# computes K/V projections for its whole batch (duplicated across the 2 cores
# sharing a batch) so no cross-core collectives are needed.
#
# Matmuls run in bf16 (fp32 PSUM accumulation); softmax statistics stay fp32.
# K/Q/V/OT all live in SBUF for the whole kernel -- no DRAM spills.
# Attention uses the transposed-scores layout St[kk, q]:
#   Kt[d, s], Qt[d, q]; St = Kt_tile.T @ Qt  (2 heads packed into one 2-bank
#   PSUM tile, exp'd in a single ACT op)
#   P = exp(St) * mask
#   OT[dv, q] += V_aug[kk, 65].T @ P   -- V carries a ones column, so PSUM
#     row 64 accumulates the softmax denominators for free.
#   OT_norm = OT * reciprocal(bcast(denoms)); y = sum_dc OT.T @ woT + b_o.
import sys

if '/opt/trn_rl_repo' not in sys.path:
    sys.path.insert(0, '/opt/trn_rl_repo')

import numpy as np

B, S, D = 4, 2048, 1024
H, DK = 16, 64
NCORES = 8
SC = 512
NKT = S // 128            # 16 kk tiles
HPN = D // 128            # 8 head-pairs
CAPS = (8, 16)            # kk-tile capacity per slot (uniform across cores)
CHUNKS = [(0, 3), (1, 2)]  # real chunk pair per core parity

_CACHE = {}


def _build_program():
    import contextlib

    import concourse.tile as tile
    from concourse import bacc, mybir

    F32 = mybir.dt.float32
    BF16 = mybir.dt.bfloat16
    EXP = mybir.ActivationFunctionType.Exp

    nc = bacc.Bacc("TRN2", target_bir_lowering=False, debug=False,
                   num_devices=NCORES)

    xT_d = nc.dram_tensor("xT", [D, S], BF16, kind="ExternalInput")
    xQT_d = nc.dram_tensor("xQT", [D, 2 * SC], BF16, kind="ExternalInput")
    wqT_d = nc.dram_tensor("wqT", [D, D], BF16, kind="ExternalInput")
    wkT_d = nc.dram_tensor("wkT", [D, D], BF16, kind="ExternalInput")
    wvT_d = nc.dram_tensor("wvT", [D, D], BF16, kind="ExternalInput")
    woT_d = nc.dram_tensor("woT", [D, D], BF16, kind="ExternalInput")
    bias_d = nc.dram_tensor("bias", [1, D], BF16, kind="ExternalInput")
    masks_d = nc.dram_tensor("masks", [128, NKT * SC], BF16,
                             kind="ExternalInput")
    y_d = nc.dram_tensor("y", [2 * SC, D], F32, kind="ExternalOutput")

    with tile.TileContext(nc) as tc, contextlib.ExitStack() as ctx:
        smalls = ctx.enter_context(tc.tile_pool(name="smalls", bufs=1))
        p_OT = ctx.enter_context(tc.tile_pool(name="otp", bufs=1))
        p_Kt = ctx.enter_context(tc.tile_pool(name="ktp", bufs=1))
        p_Qt = ctx.enter_context(tc.tile_pool(name="qtp", bufs=1))
        p_V = ctx.enter_context(tc.tile_pool(name="vp", bufs=1))
        p_mk = ctx.enter_context(tc.tile_pool(name="mk", bufs=1))

        masks_sb = p_mk.tile([128, NKT * SC], BF16, tag="masks")
        nc.gpsimd.dma_start(masks_sb[:], masks_d.ap())

        bias_sb = smalls.tile([1, D], BF16, tag="bias")
        nc.sync.dma_start(bias_sb[:], bias_d.ap())
        ones1f = smalls.tile([1, 128], F32, tag="ones1f")
        nc.vector.memset(ones1f[:], 1.0)
        ones1 = smalls.tile([1, 128], BF16, tag="ones1")
        nc.vector.tensor_copy(ones1[:], ones1f[:])
        ones256f = smalls.tile([128, 256], F32, tag="ones256f")
        nc.vector.memset(ones256f[:], 1.0)

        OT = p_OT.tile([128, HPN * 2 * SC], BF16, tag="OT")
        Kt = p_Kt.tile([128, HPN * S], BF16, tag="Kt")
        Qt = p_Qt.tile([128, HPN * 2 * SC], BF16, tag="Qt")
        Vsb = p_V.tile([128, NKT * H * 65], BF16, tag="Vsb")

        # ones columns of V_aug (all 16 s-tiles, one strided copy)
        nc.vector.tensor_copy(
            Vsb[:].rearrange("p (s h c) -> p s h c", s=NKT, c=65)
            [:, :, :, 64:65],
            ones256f[:].rearrange("p (s h) -> p s h", s=NKT)[:, :, :, None])

        # ---- V + K projections, one half of the sequence at a time ----
        with tc.tile_pool(name="xth", bufs=1) as p_xh, \
             tc.tile_pool(name="wfv", bufs=1) as p_wv, \
             tc.tile_pool(name="wfk", bufs=1) as p_wk, \
             tc.tile_pool(name="psp", bufs=8, space="PSUM") as psp:
            wv = p_wv.tile([128, 8 * D], BF16, tag="wv")
            for k in range(8):
                nc.sync.dma_start(
                    wv[:, k * D:(k + 1) * D],
                    wvT_d.ap()[k * 128:(k + 1) * 128, :])
            wk = p_wk.tile([128, 8 * D], BF16, tag="wk")
            for k in range(8):
                nc.sync.dma_start(
                    wk[:, k * D:(k + 1) * D],
                    wkT_d.ap()[k * 128:(k + 1) * 128, :])

            for half in range(2):
                xh = p_xh.tile([128, 8 * 1024], BF16, tag="xh",
                               name=f"xh_{half}")
                for k in range(8):
                    nc.sync.dma_start(
                        xh[:, k * 1024:(k + 1) * 1024],
                        xT_d.ap()[k * 128:(k + 1) * 128,
                                  half * 1024:(half + 1) * 1024])
                # V for the 8 s-tiles of this half (into SBUF V_aug layout)
                for sti in range(8):
                    st_g = half * 8 + sti
                    for dvc in range(2):
                        ps = psp.tile([128, 512], F32, tag="ps")
                        for k in range(8):
                            nc.tensor.matmul(
                                ps[:],
                                xh[:, k * 1024 + sti * 128:
                                   k * 1024 + (sti + 1) * 128],
                                wv[:, k * D + dvc * 512:k * D + (dvc + 1) * 512],
                                start=(k == 0), stop=(k == 7))
                        off = st_g * 1040 + dvc * 520
                        nc.scalar.copy(
                            Vsb[:, off:off + 520]
                            .rearrange("p (h c) -> p h c", c=65)[:, :, 0:64],
                            ps[:].rearrange("p (h c) -> p h c", c=64))
                # K for the 2 s-chunks of this half -> SBUF-resident Kt
                for sc2 in range(2):
                    sc = half * 2 + sc2
                    ps8 = [psp.tile([128, 512], F32, tag="ps",
                                    name=f"psk_{sc}_{hp}")
                           for hp in range(HPN)]
                    for k in range(8):
                        for hp in range(HPN):
                            nc.tensor.matmul(
                                ps8[hp][:],
                                wk[:, k * D + hp * 128:k * D + (hp + 1) * 128],
                                xh[:, k * 1024 + sc2 * 512:
                                   k * 1024 + (sc2 + 1) * 512],
                                start=(k == 0), stop=(k == 7))
                    for hp in range(HPN):
                        nc.vector.tensor_copy(
                            Kt[:, hp * S + sc * 512:hp * S + (sc + 1) * 512],
                            ps8[hp][:])

        # ------------- Q projection (xQT streamed, wq resident) ----------
        with tc.tile_pool(name="wf2", bufs=1) as p_w2, \
             tc.tile_pool(name="xqs", bufs=4) as p_xq, \
             tc.tile_pool(name="psq", bufs=8, space="PSUM") as psq:
            wq = p_w2.tile([128, 8 * D], BF16, tag="w2")
            for k in range(8):
                nc.sync.dma_start(
                    wq[:, k * D:(k + 1) * D],
                    wqT_d.ap()[k * 128:(k + 1) * 128, :])
            for ci in range(2):
                ps8 = [psq.tile([128, 512], F32, tag="ps",
                                name=f"psq_{ci}_{hp}") for hp in range(HPN)]
                for k in range(8):
                    xq1 = p_xq.tile([128, 512], BF16, tag="xq")
                    nc.sync.dma_start(
                        xq1[:],
                        xQT_d.ap()[k * 128:(k + 1) * 128,
                                   ci * SC:(ci + 1) * SC])
                    for hp in range(HPN):
                        nc.tensor.matmul(
                            ps8[hp][:],
                            wq[:, k * D + hp * 128:k * D + (hp + 1) * 128],
                            xq1[:], start=(k == 0), stop=(k == 7))
                for hp in range(HPN):
                    nc.scalar.copy(
                        Qt[:, hp * 2 * SC + ci * SC:
                           hp * 2 * SC + (ci + 1) * SC],
                        ps8[hp][:])

        # ---------------- attention ----------------
        with tc.tile_pool(name="rs", bufs=2) as p_rs, \
             tc.tile_pool(name="bcp", bufs=1) as p_bc, \
             tc.tile_pool(name="pp", bufs=6) as p_P, \
             tc.tile_pool(name="pst", bufs=2, space="PSUM") as p_st, \
             tc.tile_pool(name="pav", bufs=4, space="PSUM") as p_av:

            for ci, cap in enumerate(CAPS):
                for bl in range(HPN // 2):
                    av = [p_av.tile([128, 512], F32, tag="av",
                                    name=f"av_{ci}_{bl}_{i}")
                          for i in range(4)]
                    for t in range(cap):
                        for hp_i in range(2):
                            hp = 2 * bl + hp_i
                            st = p_st.tile([128, 1024], F32, tag="st")
                            for hh in range(2):
                                r0 = 64 * hh
                                nc.tensor.matmul(
                                    st[:, hh * 512:(hh + 1) * 512],
                                    Kt[r0:r0 + 64,
                                       hp * S + t * 128:hp * S + (t + 1) * 128],
                                    Qt[r0:r0 + 64,
                                       hp * 2 * SC + ci * SC:
                                       hp * 2 * SC + (ci + 1) * SC],
                                    start=True, stop=True,
                                    tile_position=(r0, 0))
                            p1 = p_P.tile([128, 1024], BF16, tag="p")
                            nc.scalar.activation(p1[:], st[:], EXP)
                            if ci == 0 or t >= 8:
                                p2 = p_P.tile([128, 1024], BF16, tag="p")
                                for hh in range(2):
                                    nc.vector.tensor_mul(
                                        p2[:, hh * 512:(hh + 1) * 512],
                                        p1[:, hh * 512:(hh + 1) * 512],
                                        masks_sb[:, t * SC:(t + 1) * SC])
                                p1 = p2
                            for hh in range(2):
                                hi = 2 * hp_i + hh
                                nc.tensor.matmul(
                                    av[hi][0:65, :],
                                    Vsb[:, t * 1040 + (2 * bl + hp_i) * 130 +
                                        hh * 65:
                                        t * 1040 + (2 * bl + hp_i) * 130 +
                                        hh * 65 + 65],
                                    p1[:, hh * 512:(hh + 1) * 512],
                                    start=(t == 0), stop=(t == cap - 1))
                    # normalize, one head-pair at a time
                    for hp_i in range(2):
                        hp = 2 * bl + hp_i
                        rs = p_rs.tile([1, 1024], F32, tag="rs")
                        for hh in range(2):
                            hi = 2 * hp_i + hh
                            nc.vector.tensor_copy(
                                rs[0:1, hh * 512:hh * 512 + 512],
                                av[hi][64:65, :])
                        bc = p_bc.tile([128, 1024], F32, tag="bc")
                        nc.gpsimd.partition_broadcast(bc[:], rs[:])
                        rbc = p_bc.tile([128, 1024], F32, tag="rbc")
                        scr = p_bc.tile([128, 1024], F32, tag="scr")
                        nc.vector.reciprocal_approx_accurate(
                            rbc[:], bc[:], scratch=scr[:])
                        for hh in range(2):
                            hi = 2 * hp_i + hh
                            r0 = 64 * hh
                            nc.vector.tensor_mul(
                                OT[r0:r0 + 64,
                                   hp * 2 * SC + ci * SC:
                                   hp * 2 * SC + (ci + 1) * SC],
                                av[hi][0:64, :],
                                rbc[r0:r0 + 64, hh * 512:hh * 512 + 512])

        # ---------------- output projection ----------------
        with tc.tile_pool(name="wo", bufs=1) as p_wo, \
             tc.tile_pool(name="ybp", bufs=4) as p_yb, \
             tc.tile_pool(name="psy", bufs=6, space="PSUM") as psy:
            wo = p_wo.tile([128, 8 * D], BF16, tag="wo")
            for k in range(8):
                nc.sync.dma_start(
                    wo[:, k * D:(k + 1) * D],
                    woT_d.ap()[k * 128:(k + 1) * 128, :])
            for qi in range(8):
                for nc2 in range(2):
                    ps = psy.tile([128, 512], F32, tag="psy")
                    for dc in range(8):
                        nc.tensor.matmul(
                            ps[:],
                            OT[:, dc * 2 * SC + qi * 128:
                               dc * 2 * SC + (qi + 1) * 128],
                            wo[:, dc * D + nc2 * 512:dc * D + (nc2 + 1) * 512],
                            start=(dc == 0), stop=False)
                    nc.tensor.matmul(
                        ps[:], ones1[:],
                        bias_sb[0:1, nc2 * 512:(nc2 + 1) * 512],
                        start=False, stop=True)
                    yb = p_yb.tile([128, 512], F32, tag="yb")
                    nc.vector.tensor_copy(yb[:], ps[:])
                    nc.sync.dma_start(
                        y_d.ap()[qi * 128:(qi + 1) * 128,
                                 nc2 * 512:(nc2 + 1) * 512], yb[:])

    nc.compile()
    return nc


def _get_program():
    if 'nc' not in _CACHE:
        _CACHE['nc'] = _build_program()
    return _CACHE['nc']


def _tri_masks():
    p = np.arange(128)[:, None]
    f = np.arange(SC)[None, :]
    return [(p <= f - 128 * r).astype(np.float32) for r in range(4)]


def _masks_for_core(c):
    import ml_dtypes
    tri = _tri_masks()
    ones = np.ones((128, SC), np.float32)
    zeros = np.zeros((128, SC), np.float32)
    j_pair = CHUNKS[c % 2]
    out = np.zeros((128, NKT * SC), np.float32)
    for ci, cap in enumerate(CAPS):
        j = j_pair[ci]
        t0 = 0 if ci == 0 else 8
        for t in range(t0, cap):
            if t < 4 * j:
                m = ones
            elif t < 4 * j + 4:
                m = tri[t - 4 * j]
            else:
                m = zeros
            out[:, t * SC:(t + 1) * SC] = m
    return out.astype(ml_dtypes.bfloat16)


def kernel(x, w_q, w_k, w_v, w_o, b_o):
    import ml_dtypes
    from concourse.bass_utils import run_bass_kernel_spmd

    BF = ml_dtypes.bfloat16
    x = np.asarray(x, dtype=np.float32)
    nc = _get_program()

    scale = np.float32(1.0 / np.sqrt(DK))
    common = {
        "wqT": np.ascontiguousarray(
            (np.asarray(w_q, np.float32).T * scale)).astype(BF),
        "wkT": np.ascontiguousarray(np.asarray(w_k, np.float32).T).astype(BF),
        "wvT": np.ascontiguousarray(np.asarray(w_v, np.float32).T).astype(BF),
        "woT": np.ascontiguousarray(np.asarray(w_o, np.float32).T).astype(BF),
        "bias": np.asarray(b_o, np.float32)[None, :].astype(BF),
    }

    in_maps = []
    for c in range(NCORES):
        b = c // 2
        j1, j2 = CHUNKS[c % 2]
        xb = x[b]
        xq = np.concatenate(
            [xb[j1 * SC:(j1 + 1) * SC], xb[j2 * SC:(j2 + 1) * SC]], axis=0)
        in_maps.append({
            "xT": np.ascontiguousarray(xb.T).astype(BF),
            "xQT": np.ascontiguousarray(xq.T).astype(BF),
            "masks": _masks_for_core(c),
            **common,
        })

    res = run_bass_kernel_spmd(nc, in_maps, core_ids=list(range(NCORES)),
                               trace=_CACHE.get('trace', False),
                               tmpdir=_CACHE.get('tmpdir'))
    _CACHE['last_res'] = res

    y = np.empty((B, S, D), dtype=np.float32)
    for c in range(NCORES):
        b = c // 2
        j1, j2 = CHUNKS[c % 2]
        yc = res.results[c]["y"]
        y[b, j1 * SC:(j1 + 1) * SC] = yc[0:SC]
        y[b, j2 * SC:(j2 + 1) * SC] = yc[SC:2 * SC]
    return y


# revision 18
# speedup vs baseline: 2.1425x; 1.0010x over previous
# Multi-head causal attention (B=4, S=2048, D=1024, H=16) on 8 TRN2 NeuronCores.
#
# Sharding: batch x query-chunk. Core c handles batch b=c//2 and two 512-row
# query chunks of that batch: cores with c%2==0 take real chunks (0, 3),
# c%2==1 take (1, 2). The SPMD program is identical on every core: it
# processes two query "slots" with fixed kk-tile capacities (8, 16); real
# chunk needs (4,8,12,16 tiles) are mapped into those capacities and the
# excess key tiles are zeroed by per-core causal-mask input data. Each core
# computes K/V projections for its whole batch (duplicated across the 2 cores
# sharing a batch) so no cross-core collectives are needed.
#
# Matmuls run in bf16 (fp32 PSUM accumulation); softmax statistics stay fp32.
# K/Q/V/OT all live in SBUF for the whole kernel -- no DRAM spills.
# Attention uses the transposed-scores layout St[kk, q]:
#   Kt[d, s], Qt[d, q]; St = Kt_tile.T @ Qt  (2 heads packed into one 2-bank
#   PSUM tile, exp'd in a single ACT op)
#   P = exp(St) * mask
#   OT[dv, q] += V_aug[kk, 65].T @ P   -- V carries a ones column, so PSUM
#     row 64 accumulates the softmax denominators for free.
#   OT_norm = OT * reciprocal(bcast(denoms)); y = sum_dc OT.T @ woT + b_o.
import sys

if '/opt/trn_rl_repo' not in sys.path:
    sys.path.insert(0, '/opt/trn_rl_repo')

import numpy as np

B, S, D = 4, 2048, 1024
H, DK = 16, 64
NCORES = 8
SC = 512
NKT = S // 128            # 16 kk tiles
HPN = D // 128            # 8 head-pairs
CAPS = (8, 16)            # kk-tile capacity per slot (uniform across cores)
CHUNKS = [(0, 3), (1, 2)]  # real chunk pair per core parity

_CACHE = {}


def _build_program():
    import contextlib

    import concourse.tile as tile
    from concourse import bacc, mybir

    F32 = mybir.dt.float32
    BF16 = mybir.dt.bfloat16
    EXP = mybir.ActivationFunctionType.Exp

    nc = bacc.Bacc("TRN2", target_bir_lowering=False, debug=False,
                   num_devices=NCORES)

    xT_d = nc.dram_tensor("xT", [D, S], BF16, kind="ExternalInput")
    xQT_d = nc.dram_tensor("xQT", [D, 2 * SC], BF16, kind="ExternalInput")
    wqT_d = nc.dram_tensor("wqT", [D, D], BF16, kind="ExternalInput")
    wkT_d = nc.dram_tensor("wkT", [D, D], BF16, kind="ExternalInput")
    wvT_d = nc.dram_tensor("wvT", [D, D], BF16, kind="ExternalInput")
    woT_d = nc.dram_tensor("woT", [D, D], BF16, kind="ExternalInput")
    bias_d = nc.dram_tensor("bias", [1, D], BF16, kind="ExternalInput")
    masks_d = nc.dram_tensor("masks", [128, NKT * 1024], BF16,
                             kind="ExternalInput")
    y_d = nc.dram_tensor("y", [2 * SC, D], F32, kind="ExternalOutput")

    with tile.TileContext(nc) as tc, contextlib.ExitStack() as ctx:
        smalls = ctx.enter_context(tc.tile_pool(name="smalls", bufs=1))
        p_OT = ctx.enter_context(tc.tile_pool(name="otp", bufs=1))
        p_Kt = ctx.enter_context(tc.tile_pool(name="ktp", bufs=1))
        p_Qt = ctx.enter_context(tc.tile_pool(name="qtp", bufs=1))
        p_V = ctx.enter_context(tc.tile_pool(name="vp", bufs=1))
        p_mk = ctx.enter_context(tc.tile_pool(name="mk", bufs=1))

        masks_sb = p_mk.tile([128, NKT * 1024], BF16, tag="masks")
        nc.gpsimd.dma_start(masks_sb[:], masks_d.ap())

        bias_sb = smalls.tile([1, D], BF16, tag="bias")
        nc.sync.dma_start(bias_sb[:], bias_d.ap())
        ones1f = smalls.tile([1, 128], F32, tag="ones1f")
        nc.vector.memset(ones1f[:], 1.0)
        ones1 = smalls.tile([1, 128], BF16, tag="ones1")
        nc.vector.tensor_copy(ones1[:], ones1f[:])
        ones256f = smalls.tile([128, 256], F32, tag="ones256f")
        nc.vector.memset(ones256f[:], 1.0)

        OT = p_OT.tile([128, HPN * 2 * SC], BF16, tag="OT")
        Kt = p_Kt.tile([128, HPN * S], BF16, tag="Kt")
        Qt = p_Qt.tile([128, HPN * 2 * SC], BF16, tag="Qt")
        Vsb = p_V.tile([128, NKT * H * 65], BF16, tag="Vsb")

        # ones columns of V_aug (all 16 s-tiles, one strided copy)
        nc.vector.tensor_copy(
            Vsb[:].rearrange("p (s h c) -> p s h c", s=NKT, c=65)
            [:, :, :, 64:65],
            ones256f[:].rearrange("p (s h) -> p s h", s=NKT)[:, :, :, None])

        # ---- V + K projections, one half of the sequence at a time ----
        with tc.tile_pool(name="xth", bufs=1) as p_xh, \
             tc.tile_pool(name="wfv", bufs=1) as p_wv, \
             tc.tile_pool(name="wfk", bufs=1) as p_wk, \
             tc.tile_pool(name="psp", bufs=8, space="PSUM") as psp:
            wv = p_wv.tile([128, 8 * D], BF16, tag="wv")
            for k in range(8):
                nc.sync.dma_start(
                    wv[:, k * D:(k + 1) * D],
                    wvT_d.ap()[k * 128:(k + 1) * 128, :])
            wk = p_wk.tile([128, 8 * D], BF16, tag="wk")
            for k in range(8):
                nc.sync.dma_start(
                    wk[:, k * D:(k + 1) * D],
                    wkT_d.ap()[k * 128:(k + 1) * 128, :])

            for half in range(2):
                xh = p_xh.tile([128, 8 * 1024], BF16, tag="xh",
                               name=f"xh_{half}")
                for k in range(8):
                    nc.sync.dma_start(
                        xh[:, k * 1024:(k + 1) * 1024],
                        xT_d.ap()[k * 128:(k + 1) * 128,
                                  half * 1024:(half + 1) * 1024])
                # V for the 8 s-tiles of this half (into SBUF V_aug layout)
                for sti in range(8):
                    st_g = half * 8 + sti
                    for dvc in range(2):
                        ps = psp.tile([128, 512], F32, tag="ps")
                        for k in range(8):
                            nc.tensor.matmul(
                                ps[:],
                                xh[:, k * 1024 + sti * 128:
                                   k * 1024 + (sti + 1) * 128],
                                wv[:, k * D + dvc * 512:k * D + (dvc + 1) * 512],
                                start=(k == 0), stop=(k == 7))
                        off = st_g * 1040 + dvc * 520
                        nc.vector.tensor_copy(
                            Vsb[:, off:off + 520]
                            .rearrange("p (h c) -> p h c", c=65)[:, :, 0:64],
                            ps[:].rearrange("p (h c) -> p h c", c=64))
                # K for the 2 s-chunks of this half -> SBUF-resident Kt
                for sc2 in range(2):
                    sc = half * 2 + sc2
                    ps8 = [psp.tile([128, 512], F32, tag="ps",
                                    name=f"psk_{sc}_{hp}")
                           for hp in range(HPN)]
                    for k in range(8):
                        for hp in range(HPN):
                            nc.tensor.matmul(
                                ps8[hp][:],
                                wk[:, k * D + hp * 128:k * D + (hp + 1) * 128],
                                xh[:, k * 1024 + sc2 * 512:
                                   k * 1024 + (sc2 + 1) * 512],
                                start=(k == 0), stop=(k == 7))
                    for hp in range(HPN):
                        nc.vector.tensor_copy(
                            Kt[:, hp * S + sc * 512:hp * S + (sc + 1) * 512],
                            ps8[hp][:])

        # ------------- Q projection (xQT streamed, wq resident) ----------
        with tc.tile_pool(name="wf2", bufs=1) as p_w2, \
             tc.tile_pool(name="xqs", bufs=4) as p_xq, \
             tc.tile_pool(name="psq", bufs=8, space="PSUM") as psq:
            wq = p_w2.tile([128, 8 * D], BF16, tag="w2")
            for k in range(8):
                nc.sync.dma_start(
                    wq[:, k * D:(k + 1) * D],
                    wqT_d.ap()[k * 128:(k + 1) * 128, :])
            for ci in range(2):
                ps8 = [psq.tile([128, 512], F32, tag="ps",
                                name=f"psq_{ci}_{hp}") for hp in range(HPN)]
                for k in range(8):
                    xq1 = p_xq.tile([128, 512], BF16, tag="xq")
                    nc.sync.dma_start(
                        xq1[:],
                        xQT_d.ap()[k * 128:(k + 1) * 128,
                                   ci * SC:(ci + 1) * SC])
                    for hp in range(HPN):
                        nc.tensor.matmul(
                            ps8[hp][:],
                            wq[:, k * D + hp * 128:k * D + (hp + 1) * 128],
                            xq1[:], start=(k == 0), stop=(k == 7))
                for hp in range(HPN):
                    nc.vector.tensor_copy(
                        Qt[:, hp * 2 * SC + ci * SC:
                           hp * 2 * SC + (ci + 1) * SC],
                        ps8[hp][:])

        # ---------------- attention ----------------
        with tc.tile_pool(name="rs", bufs=2) as p_rs, \
             tc.tile_pool(name="bcp", bufs=1) as p_bc, \
             tc.tile_pool(name="pp", bufs=8) as p_P, \
             tc.tile_pool(name="pst", bufs=2, space="PSUM") as p_st, \
             tc.tile_pool(name="pav", bufs=4, space="PSUM") as p_av:

            for ci, cap in enumerate(CAPS):
                for bl in range(HPN // 2):
                    av = [p_av.tile([128, 512], F32, tag="av",
                                    name=f"av_{ci}_{bl}_{i}")
                          for i in range(4)]

                    def emit_av(t, p_tiles, cap=cap, av=av, bl=bl):
                        for hp_i in range(2):
                            for hh in range(2):
                                hi = 2 * hp_i + hh
                                off = (t * 1040 + (2 * bl + hp_i) * 130 +
                                       hh * 65)
                                nc.tensor.matmul(
                                    av[hi][0:65, :],
                                    Vsb[:, off:off + 65],
                                    p_tiles[hp_i][:, hh * 512:(hh + 1) * 512],
                                    start=(t == 0), stop=(t == cap - 1))

                    prev = None
                    for t in range(cap):
                        p_cur = []
                        for hp_i in range(2):
                            hp = 2 * bl + hp_i
                            st = p_st.tile([128, 1024], F32, tag="st")
                            for hh in range(2):
                                r0 = 64 * hh
                                nc.tensor.matmul(
                                    st[:, hh * 512:(hh + 1) * 512],
                                    Kt[r0:r0 + 64,
                                       hp * S + t * 128:hp * S + (t + 1) * 128],
                                    Qt[r0:r0 + 64,
                                       hp * 2 * SC + ci * SC:
                                       hp * 2 * SC + (ci + 1) * SC],
                                    start=True, stop=True,
                                    tile_position=(r0, 0))
                            p1 = p_P.tile([128, 1024], BF16, tag="p")
                            nc.scalar.activation(p1[:], st[:], EXP)
                            if ci == 0 or t >= 8:
                                p2 = p_P.tile([128, 1024], BF16, tag="p")
                                nc.vector.tensor_mul(
                                    p2[:], p1[:],
                                    masks_sb[:, t * 1024:(t + 1) * 1024])
                                p1 = p2
                            p_cur.append(p1)
                        # AV for the previous tile issues now, so exp(t)
                        # overlaps the PE work of scores(t) + AV(t-1)
                        if prev is not None:
                            emit_av(prev[0], prev[1])
                        prev = (t, p_cur)
                    emit_av(prev[0], prev[1])
                    # normalize, one head-pair at a time
                    for hp_i in range(2):
                        hp = 2 * bl + hp_i
                        rs = p_rs.tile([1, 1024], F32, tag="rs")
                        for hh in range(2):
                            hi = 2 * hp_i + hh
                            nc.vector.tensor_copy(
                                rs[0:1, hh * 512:hh * 512 + 512],
                                av[hi][64:65, :])
                        bc = p_bc.tile([128, 1024], F32, tag="bc")
                        nc.gpsimd.partition_broadcast(bc[:], rs[:])
                        rbc = p_bc.tile([128, 1024], F32, tag="rbc")
                        scr = p_bc.tile([128, 1024], F32, tag="scr")
                        nc.vector.reciprocal_approx_accurate(
                            rbc[:], bc[:], scratch=scr[:])
                        for hh in range(2):
                            hi = 2 * hp_i + hh
                            r0 = 64 * hh
                            nc.vector.tensor_mul(
                                OT[r0:r0 + 64,
                                   hp * 2 * SC + ci * SC:
                                   hp * 2 * SC + (ci + 1) * SC],
                                av[hi][0:64, :],
                                rbc[r0:r0 + 64, hh * 512:hh * 512 + 512])

        # ---------------- output projection ----------------
        with tc.tile_pool(name="wo", bufs=1) as p_wo, \
             tc.tile_pool(name="ybp", bufs=4) as p_yb, \
             tc.tile_pool(name="psy", bufs=6, space="PSUM") as psy:
            wo = p_wo.tile([128, 8 * D], BF16, tag="wo")
            for k in range(8):
                nc.sync.dma_start(
                    wo[:, k * D:(k + 1) * D],
                    woT_d.ap()[k * 128:(k + 1) * 128, :])
            for qi in range(8):
                for nc2 in range(2):
                    ps = psy.tile([128, 512], F32, tag="psy")
                    for dc in range(8):
                        nc.tensor.matmul(
                            ps[:],
                            OT[:, dc * 2 * SC + qi * 128:
                               dc * 2 * SC + (qi + 1) * 128],
                            wo[:, dc * D + nc2 * 512:dc * D + (nc2 + 1) * 512],
                            start=(dc == 0), stop=False)
                    nc.tensor.matmul(
                        ps[:], ones1[:],
                        bias_sb[0:1, nc2 * 512:(nc2 + 1) * 512],
                        start=False, stop=True)
                    yb = p_yb.tile([128, 512], F32, tag="yb")
                    nc.vector.tensor_copy(yb[:], ps[:])
                    nc.sync.dma_start(
                        y_d.ap()[qi * 128:(qi + 1) * 128,
                                 nc2 * 512:(nc2 + 1) * 512], yb[:])

    nc.compile()
    return nc


def _get_program():
    if 'nc' not in _CACHE:
        _CACHE['nc'] = _build_program()
    return _CACHE['nc']


def _tri_masks():
    p = np.arange(128)[:, None]
    f = np.arange(SC)[None, :]
    return [(p <= f - 128 * r).astype(np.float32) for r in range(4)]


def _masks_for_core(c):
    import ml_dtypes
    tri = _tri_masks()
    ones = np.ones((128, SC), np.float32)
    zeros = np.zeros((128, SC), np.float32)
    j_pair = CHUNKS[c % 2]
    out = np.zeros((128, NKT * 1024), np.float32)
    for ci, cap in enumerate(CAPS):
        j = j_pair[ci]
        t0 = 0 if ci == 0 else 8
        for t in range(t0, cap):
            if t < 4 * j:
                m = ones
            elif t < 4 * j + 4:
                m = tri[t - 4 * j]
            else:
                m = zeros
            out[:, t * 1024:t * 1024 + 512] = m
            out[:, t * 1024 + 512:(t + 1) * 1024] = m
    return out.astype(ml_dtypes.bfloat16)


def kernel(x, w_q, w_k, w_v, w_o, b_o):
    import ml_dtypes
    from concourse.bass_utils import run_bass_kernel_spmd

    BF = ml_dtypes.bfloat16
    x = np.asarray(x, dtype=np.float32)
    nc = _get_program()

    scale = np.float32(1.0 / np.sqrt(DK))
    common = {
        "wqT": np.ascontiguousarray(
            (np.asarray(w_q, np.float32).T * scale)).astype(BF),
        "wkT": np.ascontiguousarray(np.asarray(w_k, np.float32).T).astype(BF),
        "wvT": np.ascontiguousarray(np.asarray(w_v, np.float32).T).astype(BF),
        "woT": np.ascontiguousarray(np.asarray(w_o, np.float32).T).astype(BF),
        "bias": np.asarray(b_o, np.float32)[None, :].astype(BF),
    }

    in_maps = []
    for c in range(NCORES):
        b = c // 2
        j1, j2 = CHUNKS[c % 2]
        xb = x[b]
        xq = np.concatenate(
            [xb[j1 * SC:(j1 + 1) * SC], xb[j2 * SC:(j2 + 1) * SC]], axis=0)
        in_maps.append({
            "xT": np.ascontiguousarray(xb.T).astype(BF),
            "xQT": np.ascontiguousarray(xq.T).astype(BF),
            "masks": _masks_for_core(c),
            **common,
        })

    res = run_bass_kernel_spmd(nc, in_maps, core_ids=list(range(NCORES)),
                               trace=_CACHE.get('trace', False),
                               tmpdir=_CACHE.get('tmpdir'))
    _CACHE['last_res'] = res

    y = np.empty((B, S, D), dtype=np.float32)
    for c in range(NCORES):
        b = c // 2
        j1, j2 = CHUNKS[c % 2]
        yc = res.results[c]["y"]
        y[b, j1 * SC:(j1 + 1) * SC] = yc[0:SC]
        y[b, j2 * SC:(j2 + 1) * SC] = yc[SC:2 * SC]
    return y
